# revision 1
# baseline (speedup 1.0000x reference)
"""Trainium2 Bass kernel for nn_CustomLoss_188978561648.

loss = -(1/K) * sum_{k,i} num[k,i] / (var + rs[k,i] - num[k,i])
  rs  = zs @ X.T          [K, N]   (the dominant GEMM)
  num = zs * diag(X)      [K, N]

Sharding: tensor-parallel over the output columns i (rows of X).
Core c owns i in [c*512, (c+1)*512): it loads X.T[:, shard] plus the
full transposed zs, computes rs[:, shard] with 32 accumulating
matmuls (contraction n on the partition axis), runs the fused
elementwise epilogue + reduction on DVE, cross-partition-reduces on
the PE against a (-1/K)-valued vector, and emits one fp32 scalar.
Host unshard = sum of the 8 per-core scalars.

Perf notes (measured on HW):
- X/zs matmul operands are cast to fp16 on the host: the loss changes
  by ~5e-7 relative (fp32 PSUM accumulation; num/den still use fp32
  data), and DMA bytes halve -> the kernel rides the ~360 GB/s HBM
  roofline at ~4.7 MB/core.
- X is transposed/packed on the host into per-chunk [zst_m | xt_m]
  interleaved blocks; the stream is 7 x 576 KB blocks + 4 chunk-level
  144 KB transfers for the last group (so the final matmuls start as
  soon as their chunk lands). All stream DMAs ride ONE Sync-engine
  HWDGE ring in FIFO program order: measured 366 GB/s with zero gaps
  (splitting across both rings or reordering measured strictly worse:
  the Scalar ring sustains only ~315 GB/s and the Tile scheduler
  reorders un-pinned DMAs ahead of gated ones).
- 7 dummy matmuls on memset data keep the PE busy through the initial
  DMA fill so the HAM clock gate reaches full speed (~1.7x matmul
  rate) before the real matmuls run; tiny (64-row) warmups measured
  too little array duty to ever un-throttle the clock.
- Epilogue at [128, 256] (full DVE width), 2-ULP approx reciprocal,
  scalar_tensor_tensor fusions incl. a fused free-axis accum_out; the
  final output is a [1, 1] scalar so its DMA uses a single engine (a
  [128, 1] output pays ~16 straggling per-engine sem completions,
  ~5 us of tail).
"""

import numpy as np

K = 64          # schedules (zs rows)
N = 4096        # channel dim
NCORES = 8
SHARD = N // NCORES            # 512 output columns per core
NCHUNKS = N // 128             # 32 contraction chunks of 128
GROUPS = 8                     # stream blocks per core
CPG = NCHUNKS // GROUPS        # 4 chunks (matmuls) per block
CCOLS = K + SHARD              # 576 cols per chunk: [zst_m | xt_m]
BCOLS = CPG * CCOLS            # 2304 block cols
EP = SHARD // 2                # 256: epilogue free size at 128 partitions
N_WARM = 7                     # PE warm-up dummy matmuls
WARM_ROWS = 512

_CACHE = {}


def _build(mm_dtype_name="float16", warm=N_WARM, fin="pe", ep="stt2"):
    import concourse.bacc as bacc
    import concourse.tile as tile
    import concourse.mybir as mybir
    f32 = mybir.dt.float32
    fmm = getattr(mybir.dt, mm_dtype_name)

    nc = bacc.Bacc(
        "TRN2", target_bir_lowering=False, debug=False, num_devices=NCORES
    )

    blk_d = nc.dram_tensor("blk", [GROUPS, 128, BCOLS], fmm, kind="ExternalInput")
    zs_d = nc.dram_tensor("zs_sh", [128, EP], f32, kind="ExternalInput")
    diag_d = nc.dram_tensor("diag", [128, EP], f32, kind="ExternalInput")
    var_d = nc.dram_tensor("var", [128, 1], f32, kind="ExternalInput")
    out_shape = [1, 1] if fin == "pe" else [128, 1]
    out_d = nc.dram_tensor("out", out_shape, f32, kind="ExternalOutput")

    with tile.TileContext(nc) as tc:
        with (
            tc.tile_pool(name="data", bufs=1) as dpool,
            tc.tile_pool(name="ep", bufs=1) as epool,
            tc.tile_pool(name="ps", bufs=1, space="PSUM") as pspool,
        ):
            # -- PE warm-up fodder (no DMA inputs) --
            dw_t = dpool.tile([128, WARM_ROWS], fmm, tag="dw")
            nc.vector.memset(dw_t[:], 0.0)
            ones_t = dpool.tile([128, 1], f32, tag="ones")
            nc.vector.memset(ones_t[:], -1.0 / K)

            # -- stream: one contiguous [zst_g | xt_g] block per group,
            #    alternating HWDGE rings; scalar ring gated on block 0 --
            # groups 0..6: one tile + one DMA per block; group 7 (the
            # critical tail) is fetched at chunk granularity so the last
            # matmuls start as soon as their 144 KB lands
            blk_t = [
                dpool.tile([128, BCOLS], fmm, name=f"blk{g}", tag=f"blk{g}")
                for g in range(GROUPS - 1)
            ]
            tail_t = [
                dpool.tile([128, CCOLS], fmm, name=f"tail{j}", tag=f"tail{j}")
                for j in range(CPG)
            ]
            for g in range(GROUPS - 1):
                nc.sync.dma_start(blk_t[g][:], blk_d[g, :, :])
            for j in range(CPG):
                nc.sync.dma_start(
                    tail_t[j][:], blk_d[GROUPS - 1, :, j * CCOLS : (j + 1) * CCOLS]
                )
            # epilogue tensors at the end of the same FIFO ring: they are
            # only needed once the last matmul group has run
            zs_t = epool.tile([128, EP], f32, tag="zs")
            diag_t = epool.tile([128, EP], f32, tag="diag")
            var_t = epool.tile([128, 1], f32, tag="var")
            nc.sync.dma_start(zs_t[:], zs_d[:])
            nc.sync.dma_start(diag_t[:], diag_d[:])
            nc.sync.dma_start(var_t[:], var_d[:])

            # -- PE: warm-up dummies, then the 32-chunk accumulation --
            if warm:
                dummy_ps = pspool.tile([K, WARM_ROWS], f32, tag="dummy_ps")
                for w in range(warm):
                    nc.tensor.matmul(
                        dummy_ps[:], dw_t[:, :K], dw_t[:], start=True, stop=True
                    )

            ps = pspool.tile([K, SHARD], f32, tag="ps")
            for g in range(GROUPS):
                for j in range(CPG):
                    m = g * CPG + j
                    src_t = blk_t[g] if g < GROUPS - 1 else tail_t[j]
                    c0 = j * CCOLS if g < GROUPS - 1 else 0
                    nc.tensor.matmul(
                        ps[:],
                        src_t[:, c0 : c0 + K],
                        src_t[:, c0 + K : c0 + CCOLS],
                        start=(m == 0),
                        stop=(m == NCHUNKS - 1),
                    )

            # -- epilogue at [128, EP]: partition p<64 -> (k=p, i<EP),
            #    p>=64 -> (k=p-64, i>=EP) --
            num_t = epool.tile([128, EP], f32, tag="num")
            nc.vector.tensor_tensor(
                num_t[:], zs_t[:], diag_t[:], op=mybir.AluOpType.mult
            )
            den_t = epool.tile([128, EP], f32, tag="den")
            rcp_t = epool.tile([128, EP], f32, tag="rcp")
            scr_t = epool.tile([128, EP], f32, tag="scr")
            red_t = epool.tile([128, 1], f32, tag="red")
            # den = (ps + var) - num
            nc.vector.scalar_tensor_tensor(
                out=den_t[:K, :], in0=ps[:, :EP], scalar=var_t[:K],
                in1=num_t[:K, :],
                op0=mybir.AluOpType.add, op1=mybir.AluOpType.subtract,
            )
            nc.vector.scalar_tensor_tensor(
                out=den_t[K:, :], in0=ps[:, EP:], scalar=var_t[K:],
                in1=num_t[K:, :],
                op0=mybir.AluOpType.add, op1=mybir.AluOpType.subtract,
            )
            nc.vector.reciprocal_approx_accurate(rcp_t[:], den_t[:], scr_t[:])
            if ep == "stt2":
                # scr = num * rcp; red = sum_free(scr), one DVE pass
                nc.vector.scalar_tensor_tensor(
                    out=scr_t[:], in0=num_t[:], scalar=1.0, in1=rcp_t[:],
                    op0=mybir.AluOpType.mult, op1=mybir.AluOpType.mult,
                    accum_out=red_t[:],
                )
            else:
                nc.vector.tensor_tensor(
                    scr_t[:], num_t[:], rcp_t[:], op=mybir.AluOpType.mult
                )
                nc.vector.tensor_reduce(
                    red_t[:], scr_t[:], axis=mybir.AxisListType.X,
                    op=mybir.AluOpType.add,
                )
            if fin == "pe":
                # cross-partition reduce on PE: out = red.T @ (-1/K * ones)
                ps1 = pspool.tile([1, 1], f32, tag="ps1")
                nc.tensor.matmul(ps1[:], red_t[:], ones_t[:], start=True, stop=True)
                out_sb = epool.tile([1, 1], f32, tag="out_sb")
                nc.vector.tensor_copy(out_sb[:], ps1[:])
                nc.scalar.dma_start(out_d[:], out_sb[:])
            else:
                nc.vector.tensor_scalar_mul(red_t[:], red_t[:], -1.0 / K)
                nc.scalar.dma_start(out_d[:], red_t[:])

    nc.compile()
    return nc


def _prep_inputs(zs, X, var_noise, mm_dtype_name="float16"):
    """Host-side shard + layout packing (layout + dtype cast only; the
    only math is extracting diag(X))."""
    np_mm = {"float16": np.float16, "bfloat16": None, "float32r": np.float32,
             "float32": np.float32}[mm_dtype_name]
    if np_mm is None:
        import ml_dtypes
        np_mm = ml_dtypes.bfloat16
    zs = np.ascontiguousarray(np.asarray(zs, dtype=np.float32))
    X = np.ascontiguousarray(np.asarray(X, dtype=np.float32))
    var = np.float32(np.asarray(var_noise).reshape(()))

    # per contraction chunk m: zc[m, p, k] = zs[k, m*128 + p] (replicated),
    # xc[c, m, p, il] = X[c*SHARD + il, m*128 + p]
    zc = zs.reshape(K, NCHUNKS, 128).transpose(1, 2, 0).astype(np_mm)
    xc = X.reshape(NCORES, SHARD, NCHUNKS, 128).transpose(0, 2, 3, 1).astype(np_mm)

    diag = np.ascontiguousarray(np.diagonal(X))
    var_tile = np.full((128, 1), var, dtype=np.float32)

    def fold(a):  # [K, SHARD] -> [128, EP] epilogue layout
        return np.ascontiguousarray(np.concatenate([a[:, :EP], a[:, EP:]], axis=0))

    in_maps = []
    for c in range(NCORES):
        sl = slice(c * SHARD, (c + 1) * SHARD)
        zs_sh = zs[:, sl]
        diag_bc = np.broadcast_to(diag[sl], (K, SHARD))
        in_maps.append(
            {
                "blk": np.ascontiguousarray(
                    np.concatenate([zc, xc[c]], axis=-1)  # [32, 128, 576]
                    .reshape(GROUPS, CPG, 128, CCOLS)
                    .transpose(0, 2, 1, 3)
                ).reshape(GROUPS, 128, BCOLS),
                "zs_sh": fold(zs_sh),
                "diag": fold(diag_bc),
                "var": var_tile,
            }
        )
    return in_maps


def _run(in_maps, mm_dtype_name="float16", warm=N_WARM, fin="pe", ep="stt2",
         **run_kwargs):
    from concourse.bass_utils import run_bass_kernel_spmd

    key = ("nc", mm_dtype_name, warm, fin, ep)
    if key not in _CACHE:
        _CACHE[key] = _build(mm_dtype_name, warm=warm, fin=fin, ep=ep)
    nc = _CACHE[key]
    return run_bass_kernel_spmd(
        nc, in_maps, core_ids=list(range(NCORES)), **run_kwargs
    )


def kernel(zs, X, var_noise):
    in_maps = _prep_inputs(zs, X, var_noise)
    res = None
    for attempt in range(3):
        try:
            res = _run(in_maps).results
            break
        except Exception:
            if attempt == 2:
                raise
            import time

            time.sleep(2)
    total = np.float32(0.0)
    for c in range(NCORES):
        total += res[c]["out"].astype(np.float32).sum(dtype=np.float32)
    return np.float32(total)



# revision 2
# speedup vs baseline: 1.3638x; 1.3638x over previous
"""Trainium2 Bass kernel for nn_CustomLoss_188978561648.

loss = -(1/K) * sum_{k,i} num[k,i] / (var + rs[k,i] - num[k,i])
  rs  = zs @ X.T          [K, N]   (the dominant GEMM)
  num = zs * diag(X)      [K, N]

Sharding: tensor-parallel over the output columns i (rows of X).
Core c owns i in [c*512, (c+1)*512).

v2 design (vs the 32 us fp16 baseline):
- fp8e4 (e4m3) matmul operands + MatmulPerfMode.DoubleRow: halves both
  the HBM stream (2.3 MB/core) and the PE column count (16 matmuls,
  each contracting 256 rows). Measured host-side rel err ~1.6e-5 vs
  the 2e-2 gate (random quantization noise averages out over the
  4096-term contraction).
- X is diag-zeroed on the host, so the GEMM computes rs - num
  directly; +var is folded in as a rank-1 fp16 matmul (lhsT [1,64] of
  var, rhs [1,512] of ones) that runs at stream start. PSUM then holds
  the full denominator, so the epilogue is just
  rcp_approx_fast + one STT (num * rcp, fused free-axis accum).
- num = zs*diag shipped fp16; final cross-partition reduce on the PE
  against a (-1/K) ones vector; [1,1] fp32 output on the scalar ring.
"""

import numpy as np

K = 64          # schedules (zs rows)
N = 4096        # channel dim
NCORES = 8
SHARD = N // NCORES            # 512 output columns per core
NCHUNKS = N // 128             # 32 contraction chunks of 128
NPAIRS = NCHUNKS // 2          # 16 DoubleRow chunk pairs
PAIR_BLOCKS = (4, 4, 4, 2, 2)  # xt stream granularity (pairs per DMA)
XCOLS = NCHUNKS * SHARD        # 16384 packed xt cols per partition

_CACHE = {}


def _build():
    import concourse.bacc as bacc
    import concourse.tile as tile
    import concourse.mybir as mybir
    f32 = mybir.dt.float32
    f16 = mybir.dt.float16
    f8 = mybir.dt.float8e4

    nc = bacc.Bacc(
        "TRN2", target_bir_lowering=False, debug=False, num_devices=NCORES
    )

    varrow_d = nc.dram_tensor("varrow", [1, K + SHARD], f16, kind="ExternalInput")
    zst_d = nc.dram_tensor("zst", [128, NCHUNKS * K], f8, kind="ExternalInput")
    xt_d = nc.dram_tensor("xt", [128, XCOLS], f8, kind="ExternalInput")
    num_d = nc.dram_tensor("num", [K, SHARD], f16, kind="ExternalInput")
    out_d = nc.dram_tensor("out", [1, 1], f32, kind="ExternalOutput")

    with tile.TileContext(nc) as tc:
        with (
            tc.tile_pool(name="data", bufs=1) as dpool,
            tc.tile_pool(name="ep", bufs=1) as epool,
            tc.tile_pool(name="ps", bufs=1, space="PSUM") as pspool,
        ):
            ones_t = epool.tile([K, 1], f32, tag="ones")
            nc.vector.memset(ones_t[:], -1.0 / K)

            # -- stream: varrow + zst up front, then xt pair-blocks, num last.
            #    All on the sync HWDGE ring in FIFO program order. --
            varrow_t = dpool.tile([1, K + SHARD], f16, tag="varrow")
            nc.sync.dma_start(varrow_t[:], varrow_d[:])
            zst_t = dpool.tile([128, NCHUNKS, K], f8, tag="zst")
            nc.sync.dma_start(zst_t[:], zst_d[:])
            xt_t = []
            off = 0
            for b, npair in enumerate(PAIR_BLOCKS):
                cols = npair * 2 * SHARD
                t = dpool.tile([128, npair * 2, SHARD], f8, tag=f"xt{b}")
                nc.sync.dma_start(t[:], xt_d[:, off : off + cols])
                xt_t.append(t)
                off += cols
            num_t = epool.tile([K, SHARD], f16, tag="num")
            nc.sync.dma_start(num_t[:], num_d[:])

            # -- PE: +var rank-1 matmul, then 16 fp8 DoubleRow pair matmuls --
            ps = pspool.tile([K, SHARD], f32, tag="ps")
            nc.tensor.matmul(
                ps[:],
                varrow_t[:, :K],
                varrow_t[:, K:],
                start=True,
                stop=False,
                skip_group_check=True,
            )
            j = 0
            for b, npair in enumerate(PAIR_BLOCKS):
                for jj in range(npair):
                    nc.tensor.matmul(
                        ps[:],
                        zst_t[:, 2 * j : 2 * j + 2, :],
                        xt_t[b][:, 2 * jj : 2 * jj + 2, :],
                        start=False,
                        stop=(j == NPAIRS - 1),
                        perf_mode=mybir.MatmulPerfMode.DoubleRow,
                        skip_group_check=True,
                    )
                    j += 1

            # -- epilogue: PSUM already holds den = var + rs - num --
            rcp_t = epool.tile([K, SHARD], f32, tag="rcp")
            scr_t = epool.tile([K, SHARD], f32, tag="scr")
            red_t = epool.tile([K, 1], f32, tag="red")
            nc.vector.reciprocal_approx_fast(rcp_t[:], ps[:])
            nc.vector.scalar_tensor_tensor(
                out=scr_t[:], in0=num_t[:], scalar=1.0, in1=rcp_t[:],
                op0=mybir.AluOpType.mult, op1=mybir.AluOpType.mult,
                accum_out=red_t[:],
            )
            # cross-partition reduce on PE: out = red.T @ (-1/K * ones)
            ps1 = pspool.tile([1, 1], f32, tag="ps1")
            nc.tensor.matmul(ps1[:], red_t[:], ones_t[:], start=True, stop=True)
            out_sb = epool.tile([1, 1], f32, tag="out_sb")
            nc.vector.tensor_copy(out_sb[:], ps1[:])
            nc.scalar.dma_start(out_d[:], out_sb[:])

    nc.compile()
    return nc


def _prep_inputs(zs, X, var_noise):
    """Host-side shard + layout packing (layout + dtype cast; the only
    math is diag extraction, the zs*diag elementwise product, and
    zeroing X's diagonal)."""
    import ml_dtypes

    f8 = ml_dtypes.float8_e4m3
    zs = np.ascontiguousarray(np.asarray(zs, dtype=np.float32))
    X = np.asarray(X, dtype=np.float32)
    var = np.float32(np.asarray(var_noise).reshape(()))

    diag = np.ascontiguousarray(np.diagonal(X)).astype(np.float32)
    Xz = X.copy()
    np.fill_diagonal(Xz, 0.0)

    # zst[p, m, k] = zs[k, 128m + p], replicated across cores
    zst = np.ascontiguousarray(
        zs.reshape(K, NCHUNKS, 128).transpose(2, 1, 0)
    ).astype(f8).reshape(128, NCHUNKS * K)

    varrow = np.empty((1, K + SHARD), dtype=np.float16)
    varrow[0, :K] = var
    varrow[0, K:] = 1.0

    in_maps = []
    for c in range(NCORES):
        sl = slice(c * SHARD, (c + 1) * SHARD)
        # xt[p, m, il] = Xz[c*512 + il, 128m + p]
        xt = np.ascontiguousarray(
            Xz[sl].reshape(SHARD, NCHUNKS, 128).transpose(2, 1, 0)
        ).astype(f8).reshape(128, XCOLS)
        num = (zs[:, sl] * diag[sl][None, :]).astype(np.float16)
        in_maps.append({"varrow": varrow, "zst": zst, "xt": xt, "num": num})
    return in_maps


def _run(in_maps, **run_kwargs):
    from concourse.bass_utils import run_bass_kernel_spmd

    if "nc" not in _CACHE:
        _CACHE["nc"] = _build()
    nc = _CACHE["nc"]
    return run_bass_kernel_spmd(
        nc, in_maps, core_ids=list(range(NCORES)), **run_kwargs
    )


def kernel(zs, X, var_noise):
    in_maps = _prep_inputs(zs, X, var_noise)
    res = None
    for attempt in range(3):
        try:
            res = _run(in_maps).results
            break
        except Exception:
            if attempt == 2:
                raise
            import time

            time.sleep(2)
    total = np.float32(0.0)
    for c in range(NCORES):
        total += res[c]["out"].astype(np.float32).sum(dtype=np.float32)
    return np.float32(total)
